# revision 4
# baseline (speedup 1.0000x reference)
"""GCN 2-layer message-passing block on 8 Trainium2 NeuronCores.

Collapsed algebra (validated against the jax reference to 7e-7 in fp64):
  dis = deg^-0.5 (deg over edge sources), x~ = dis * x          (host)
  a[d] = sum_{e->d} dis[row_e];  c = dis*a                      (host)
  c2   = dis * A(dis*c);  W12 = W2@W1;  v = W2@b1               (host)
  g1[u]  = sum_{e: col=u} x~[row_e]      -- aggregation 1 (device)
  tab1   = dis^2 * g1                    -- scale (device)
  g2[d]  = sum_{e->d} tab1[row_e]        -- aggregation 2 (device)
  y2     = (dis*g2) @ W12.T + c2 (x) v + c (x) b2   -- one Lin + rank-1s

Sharding: destinations split into 8 blocks of 12500 nodes; zero-communication
(each core re-derives tab1 at S_c = distinct sources of its own edges).
Aggregations are matmuls with static one-hot block matrices in fp32 PSUM;
the inter-stage shuffle uses the custom dma_gather.
"""
import sys

sys.path.insert(0, "/opt/trn_rl_repo")

import numpy as np
import ml_dtypes

BF16 = ml_dtypes.bfloat16

N_NODES = 100000
N_EDGES = 200000
H = 384
KB = H // 128
M_CORES = 8
NPC = N_NODES // M_CORES
NTB = (NPC + 127) // 128       # 98
NPC_PAD = NTB * 128            # 12544
GA = 4                          # table-write batching (stage A)
GB = 7                          # table-write batching (stages C/D); 98 % 7 == 0


def _pack_tokens(dest_local, ntiles, blocks_per_tile):
    order = np.argsort(dest_local // 128, kind="stable")
    tile_id = dest_local[order] // 128
    tile_start = np.searchsorted(tile_id, np.arange(ntiles))
    rank = np.arange(dest_local.size) - tile_start[tile_id]
    slot = tile_id * (blocks_per_tile * 128) + rank
    return order, slot


def _wrap_idx(idx):
    n = idx.size
    w = idx.reshape(n // 16, 16).T.astype(np.int16)
    return np.tile(w, (8, 1))


def _pm(tokens, ntiles, bpt, width):
    """[ntiles*bpt*128, width] -> partition-major [ntiles, 128, bpt, width]."""
    return np.ascontiguousarray(
        tokens.reshape(ntiles, bpt, 128, width).transpose(0, 2, 1, 3))


def _prep(x, edge_index, W1, b1, W2, b2):
    row = np.asarray(edge_index[0], dtype=np.int64)
    col = np.asarray(edge_index[1], dtype=np.int64)
    xf = np.asarray(x, dtype=np.float64)

    deg = np.bincount(row, minlength=N_NODES).astype(np.float64)
    dis = deg ** -0.5
    a = np.bincount(col, weights=dis[row], minlength=N_NODES)
    cvec = dis * a
    c2 = dis * np.bincount(col, weights=(dis * cvec)[row], minlength=N_NODES)
    W12 = np.asarray(W2, np.float64) @ np.asarray(W1, np.float64)
    vv = np.asarray(W2, np.float64) @ np.asarray(b1, np.float64)
    xt = (dis[:, None] * xf).astype(BF16)

    core_of = col // NPC

    metas = []
    NTA = 0
    B1 = 0
    B2 = 0
    for cc in range(M_CORES):
        em = core_of == cc
        er, ec = row[em], col[em]
        S = np.unique(er)
        pos = np.full(N_NODES, -1, dtype=np.int64)
        pos[S] = np.arange(S.size)
        d2_all = pos[col]
        e2m = d2_all >= 0
        d2, r2 = d2_all[e2m], row[e2m]
        nta = (S.size + 127) // 128
        cntA = np.bincount(d2 // 128, minlength=nta)
        cntB = np.bincount((ec - cc * NPC) // 128, minlength=NTB)
        NTA = max(NTA, nta)
        B1 = max(B1, int(-(-cntA.max() // 128)))
        B2 = max(B2, int(-(-cntB.max() // 128)))
        metas.append((er, ec, S, pos, d2, r2))

    w12 = np.ascontiguousarray(
        W12.T.astype(BF16).reshape(KB, 128, H).transpose(1, 0, 2))
    vrow = vv.astype(BF16).reshape(1, H)
    b2r = np.asarray(b2, dtype=BF16).reshape(1, H)

    in_maps = []
    for cc in range(M_CORES):
        er, ec, S, pos, d2, r2 = metas[cc]
        n1 = NTA * B1 * 128
        n2 = NTB * B2 * 128

        orderA, slotA = _pack_tokens(d2, NTA, B1)
        t1 = np.zeros((n1, H), dtype=BF16)
        t1[slotA] = xt[r2[orderA]]
        s1 = np.zeros((NTA * B1, 128, 128), dtype=BF16)
        s1[slotA // 128, slotA % 128, d2[orderA] % 128] = 1.0

        tmp = np.zeros(NTA * 128, dtype=np.float32)
        tmp[: S.size] = (dis[S] ** 2).astype(np.float32)
        dis2arr = np.ascontiguousarray(tmp.reshape(NTA, 128).T)

        dl = ec - cc * NPC
        orderB, slotB = _pack_tokens(dl, NTB, B2)
        gidx = np.zeros(n2, dtype=np.int64)
        gidx[slotB] = pos[er[orderB]]
        s2 = np.zeros((NTB * B2, 128, 128), dtype=BF16)
        s2[slotB // 128, slotB % 128, dl[orderB] % 128] = 1.0

        tmp = np.zeros(NPC_PAD, dtype=np.float32)
        tmp[:NPC] = dis[cc * NPC : (cc + 1) * NPC].astype(np.float32)
        disarr = np.ascontiguousarray(tmp.reshape(NTB, 128).T)

        crow = np.zeros((1, NPC_PAD), dtype=BF16)
        crow[0, :NPC] = cvec[cc * NPC : (cc + 1) * NPC].astype(BF16)
        c2row = np.zeros((1, NPC_PAD), dtype=BF16)
        c2row[0, :NPC] = c2[cc * NPC : (cc + 1) * NPC].astype(BF16)

        in_maps.append({
            "t1": _pm(t1, NTA, B1, H),
            "s1": _pm(s1.reshape(NTA * B1 * 128, 128), NTA, B1, 128),
            "dis2": dis2arr,
            "gidx": _wrap_idx(gidx),
            "s2": _pm(s2.reshape(NTB * B2 * 128, 128), NTB, B2, 128),
            "disc": disarr,
            "iidx": _wrap_idx(np.arange(NPC_PAD)),
            "crow": crow, "c2row": c2row,
            "w12": w12, "vrow": vrow, "b2r": b2r,
        })
    return in_maps, NTA, B1, B2


def _build(NTA, B1, B2):
    import concourse.bass as bass
    import concourse.bacc as bacc
    import concourse.mybir as mybir
    import concourse.tile as tile

    dt = mybir.dt
    AF = mybir.ActivationFunctionType
    n2 = NTB * B2 * 128

    nc = bacc.Bacc(None, target_bir_lowering=False)
    t1 = nc.dram_tensor("t1", [NTA, 128, B1, H], dt.bfloat16, kind="ExternalInput")
    s1 = nc.dram_tensor("s1", [NTA, 128, B1, 128], dt.bfloat16, kind="ExternalInput")
    dis2 = nc.dram_tensor("dis2", [128, NTA], dt.float32, kind="ExternalInput")
    gidx = nc.dram_tensor("gidx", [128, n2 // 16], dt.int16, kind="ExternalInput")
    s2 = nc.dram_tensor("s2", [NTB, 128, B2, 128], dt.bfloat16, kind="ExternalInput")
    disc = nc.dram_tensor("disc", [128, NTB], dt.float32, kind="ExternalInput")
    iidx = nc.dram_tensor("iidx", [128, NPC_PAD // 16], dt.int16, kind="ExternalInput")
    crow = nc.dram_tensor("crow", [1, NPC_PAD], dt.bfloat16, kind="ExternalInput")
    c2row = nc.dram_tensor("c2row", [1, NPC_PAD], dt.bfloat16, kind="ExternalInput")
    w12 = nc.dram_tensor("w12", [128, KB, H], dt.bfloat16, kind="ExternalInput")
    vrow = nc.dram_tensor("vrow", [1, H], dt.bfloat16, kind="ExternalInput")
    b2r = nc.dram_tensor("b2r", [1, H], dt.bfloat16, kind="ExternalInput")
    tab1 = nc.dram_tensor("tab1", [NTA * 128, H], dt.bfloat16, kind="Internal")
    ztab = nc.dram_tensor("ztab", [NPC_PAD, H], dt.bfloat16, kind="Internal")
    out = nc.dram_tensor("out", [NPC_PAD, H], dt.float32, kind="ExternalOutput")

    with tile.TileContext(nc) as tc:
        with (
            tc.tile_pool(name="const", bufs=1) as cp,
            tc.tile_pool(name="io", bufs=3) as iop,
            tc.tile_pool(name="stg", bufs=2) as stgp,
            tc.tile_pool(name="ps", bufs=2, space="PSUM") as psp,
        ):
            w12_sb = cp.tile([128, KB, H], dt.bfloat16)
            nc.sync.dma_start(w12_sb[:], w12[:])
            vrow_sb = cp.tile([1, H], dt.bfloat16)
            nc.sync.dma_start(vrow_sb[:], vrow[:])
            b2r_sb = cp.tile([1, H], dt.bfloat16)
            nc.sync.dma_start(b2r_sb[:], b2r[:])
            dis2_sb = cp.tile([128, NTA], dt.float32)
            nc.sync.dma_start(dis2_sb[:], dis2[:])
            disc_sb = cp.tile([128, NTB], dt.float32)
            nc.sync.dma_start(disc_sb[:], disc[:])
            gidx_sb = cp.tile([128, n2 // 16], dt.int16)
            nc.sync.dma_start(gidx_sb[:], gidx[:])
            iidx_sb = cp.tile([128, NPC_PAD // 16], dt.int16)
            nc.sync.dma_start(iidx_sb[:], iidx[:])
            crow_sb = cp.tile([1, NPC_PAD], dt.bfloat16)
            nc.sync.dma_start(crow_sb[:], crow[:])
            c2row_sb = cp.tile([1, NPC_PAD], dt.bfloat16)
            nc.sync.dma_start(c2row_sb[:], c2row[:])

            # ---------- stage A: tab1 = dis^2 * (S1 @ t1) ----------
            i0 = 0
            while i0 < NTA:
                ga = min(GA, NTA - i0)
                tws = stgp.tile([128, GA, H], dt.bfloat16, tag="tws")
                for g in range(ga):
                    i = i0 + g
                    t1_sb = iop.tile([128, B1, H], dt.bfloat16, tag="t1")
                    nc.sync.dma_start(t1_sb[:], t1[i])
                    s1_sb = iop.tile([128, B1, 128], dt.bfloat16, tag="s1")
                    nc.scalar.dma_start(s1_sb[:], s1[i])
                    psA = psp.tile([128, H], dt.float32, tag="psA")
                    for b in range(B1):
                        nc.tensor.matmul(psA[:], s1_sb[:, b, :], t1_sb[:, b, :],
                                         start=(b == 0), stop=(b == B1 - 1))
                    if i % 2 == 0:
                        nc.vector.tensor_scalar_mul(
                            tws[:, g, :], psA[:], dis2_sb[:, i:i + 1])
                    else:
                        nc.scalar.activation(
                            tws[:, g, :], psA[:], AF.Copy,
                            scale=dis2_sb[:, i:i + 1])
                nc.sync.dma_start(
                    tab1[i0 * 128:(i0 + ga) * 128, :]
                    .rearrange("(g p) h -> p g h", p=128),
                    tws[:, :ga, :])
                i0 += ga

            # ---------- stage B/C: ztab = dis * (S2 @ tab1[gidx]) ----------
            for j0 in range(0, NTB, GB):
                zws = stgp.tile([128, GB, H], dt.bfloat16, tag="zws")
                for g in range(GB):
                    j = j0 + g
                    g_sb = iop.tile([128, B2, H], dt.bfloat16, tag="g")
                    nc.gpsimd.dma_gather(
                        g_sb[:], tab1[:, :],
                        gidx_sb[:, j * (B2 * 8):(j + 1) * (B2 * 8)],
                        B2 * 128, B2 * 128, H, transpose=False)
                    psC = psp.tile([128, H], dt.float32, tag="psC")
                    s2_sb = iop.tile([128, B2, 128], dt.bfloat16, tag="s2")
                    nc.scalar.dma_start(s2_sb[:], s2[j])
                    for b in range(B2):
                        nc.tensor.matmul(psC[:], s2_sb[:, b, :], g_sb[:, b, :],
                                         start=(b == 0), stop=(b == B2 - 1))
                    if j % 2 == 0:
                        nc.vector.tensor_scalar_mul(
                            zws[:, g, :], psC[:], disc_sb[:, j:j + 1])
                    else:
                        nc.scalar.activation(
                            zws[:, g, :], psC[:], AF.Copy,
                            scale=disc_sb[:, j:j + 1])
                nc.sync.dma_start(
                    ztab[j0 * 128:(j0 + GB) * 128, :]
                    .rearrange("(g p) h -> p g h", p=128),
                    zws[:])

            # ---------- stage D: out = ztab^T-gathered @ W12T + c2 x v + c x b2 --
            for j0 in range(0, NTB, GB):
                ows = stgp.tile([128, GB, H], dt.float32, tag="ows")
                for g in range(GB):
                    j = j0 + g
                    zg = iop.tile([128, KB, 128], dt.bfloat16, tag="zg")
                    nc.gpsimd.dma_gather(
                        zg[:], ztab[:, :], iidx_sb[:, j * 8:(j + 1) * 8],
                        128, 128, H, transpose=True)
                    psD = psp.tile([128, H], dt.float32, tag="psD")
                    for k in range(KB):
                        nc.tensor.matmul(psD[:], zg[:, k, :], w12_sb[:, k, :],
                                         start=(k == 0), stop=False)
                    nc.tensor.matmul(psD[:], c2row_sb[:, j * 128:(j + 1) * 128],
                                     vrow_sb[:], start=False, stop=False)
                    nc.tensor.matmul(psD[:], crow_sb[:, j * 128:(j + 1) * 128],
                                     b2r_sb[:], start=False, stop=True)
                    if j % 2 == 0:
                        nc.vector.tensor_copy(ows[:, g, :], psD[:])
                    else:
                        nc.scalar.activation(ows[:, g, :], psD[:], AF.Copy)
                nc.sync.dma_start(
                    out[j0 * 128:(j0 + GB) * 128, :]
                    .rearrange("(g p) h -> p g h", p=128),
                    ows[:])

    nc.compile()
    return nc


_CACHE = {}


def kernel(x, edge_index, W1, b1, W2, b2):
    from concourse import bass_utils

    in_maps, NTA, B1, B2 = _prep(x, edge_index, W1, b1, W2, b2)
    key = (NTA, B1, B2)
    if key not in _CACHE:
        _CACHE[key] = _build(NTA, B1, B2)
    nc = _CACHE[key]
    res = bass_utils.run_bass_kernel_spmd(nc, in_maps, core_ids=list(range(M_CORES)))
    outs = [np.asarray(res.results[i]["out"][:NPC]) for i in range(M_CORES)]
    return np.concatenate(outs, axis=0).astype(np.float32)
